# revision 2
# baseline (speedup 1.0000x reference)
"""Trainium2 Bass kernel v5 for nn_ClassificationLoss (BCE-with-logits +
graph Laplacian regularizer), data-parallel over 8 NeuronCores.

loss = mean(softplus(logits) - targets*logits)
       + 1e-4 * 0.5 * sum_e ||params[parent_e] - params[child_e]||^2

Design (per core, all inputs fp8 e4m3 converted on host; HW-verified op set:
tensor_tensor fp8 on DVE, tensor_scalar+accum, GPSIMD tensor_tensor with
matching in/out dtype, fp8 dma_gather; tensor_tensor_reduce is NOT used --
it faults this hardware):
  - BCE rows [256c, 256c+256): 4 chunks of [128, 5000]. softplus via ACT
    Exp then Ln(bias=1, accum_out), all Exps grouped before all Lns so the
    activation-table loads collapse to one per set. t*x = DVE
    tensor_tensor mult (fp8 -> bf16) + tensor_scalar mult-by-1 with
    accum_out (2x on the bf16 product).
  - Regularizer edges [2500c, 2500c+2500) padded to 2560, gathered fp8 in
    two chunks per side (4+16 of 20 gather columns). d = gp - gc on GPSIMD
    (fp8 in/out -- dtype conversion on GPSIMD faults). sum(d^2): chunks
    0-1 on DVE (mult + ts-accum), chunks 2-4 on ACT (Square + accum_out,
    same table set as Ln).
  - Partial sums land in a [128, 16] f32 tensor, host-reduced in f64.
"""
import os
import sys

import numpy as np
import ml_dtypes

for _p in ("/opt/trn_rl_repo", "/root/.axon_site/_ro/trn_rl_repo"):
    if os.path.isdir(_p) and _p not in sys.path:
        sys.path.append(_p)

from contextlib import ExitStack

import concourse.bass as bass
import concourse.tile as tile
from concourse import bacc, mybir
from concourse.bass_utils import run_bass_kernel_spmd

fp8 = ml_dtypes.float8_e4m3
AF = mybir.ActivationFunctionType

N_CORES = 8
BATCH, N_LABELS, HIDDEN, N_EDGES = 2048, 10000, 768, 20000
PENALTY = 1e-4
ROWS = BATCH // N_CORES            # 256 rows per core
BLOCKS = ROWS // 128               # 2 partition blocks
NCH = 2                            # bce col-chunks per block
CHUNK = N_LABELS // NCH            # 5000 (640 KB per fp8 DMA)
EDGES_PC = N_EDGES // N_CORES      # 2500 edges per core
EDGES_PAD = 2560                   # padded to 20*128
GCOLS = EDGES_PAD // 128           # 20 gather columns
GSPLIT = 4                         # gather chunk 0 covers cols [0,4)
RCH = 5                            # reg compute chunks (4 cols each)
RCOLS = GCOLS // RCH
N_DVE_SQ = 2                       # reg chunks whose square runs on DVE
NBCE = BLOCKS * NCH                # 4 bce chunks
# partials columns: [0:4) softplus sums, [4:8) t*x sums, [8:13) reg sums
P_COLS = 16

_cache = {}


def _build_nc():
    nc = bacc.Bacc("TRN2", target_bir_lowering=False, debug=False,
                   num_devices=N_CORES)
    with tile.TileContext(nc) as tc, ExitStack() as ctx:
        io_pool = ctx.enter_context(tc.tile_pool(name="io", bufs=4))
        act_pool = ctx.enter_context(tc.tile_pool(name="act", bufs=2))
        ex_pool = ctx.enter_context(tc.tile_pool(name="ex", bufs=1))
        g_pool = ctx.enter_context(tc.tile_pool(name="g", bufs=1))
        d_pool = ctx.enter_context(tc.tile_pool(name="d", bufs=4))

        logits_d = nc.dram_tensor(
            "logits", [BLOCKS, 128, N_LABELS], mybir.dt.float8e4, kind="ExternalInput")
        targets_d = nc.dram_tensor(
            "targets", [BLOCKS, 128, N_LABELS], mybir.dt.float8e4, kind="ExternalInput")
        params_d = nc.dram_tensor(
            "params", [N_LABELS, HIDDEN], mybir.dt.float8e4, kind="ExternalInput")
        idxp_d = nc.dram_tensor(
            "idxp", [128, EDGES_PAD // 16], mybir.dt.int16, kind="ExternalInput")
        idxc_d = nc.dram_tensor(
            "idxc", [128, EDGES_PAD // 16], mybir.dt.int16, kind="ExternalInput")
        out_d = nc.dram_tensor(
            "partials", [128, P_COLS], mybir.dt.float32, kind="ExternalOutput")

        parts = g_pool.tile([128, P_COLS], mybir.dt.float32)
        nc.vector.memset(parts[:], 0.0)

        # edge index loads first on the scalar ring (small)
        itp = g_pool.tile([128, EDGES_PAD // 16], mybir.dt.int16)
        itc = g_pool.tile([128, EDGES_PAD // 16], mybir.dt.int16)
        nc.scalar.dma_start(out=itp[:], in_=idxp_d[:])
        nc.scalar.dma_start(out=itc[:], in_=idxc_d[:])

        # --- regularizer gathers: 2 chunks per side (SWDGE) ---
        gp = g_pool.tile([128, GCOLS * HIDDEN], mybir.dt.float8e4)
        gc = g_pool.tile([128, GCOLS * HIDDEN], mybir.dt.float8e4)
        gp3 = gp[:].rearrange("p (c s) -> p c s", s=HIDDEN)
        gc3 = gc[:].rearrange("p (c s) -> p c s", s=HIDDEN)
        for lo, hi in ((0, GSPLIT), (GSPLIT, GCOLS)):
            n = (hi - lo) * 128
            nc.gpsimd.dma_gather(
                gp3[:, lo:hi, :], params_d[:],
                itp[:, lo * 8:hi * 8], n, n, HIDDEN, single_packet=False)
            nc.gpsimd.dma_gather(
                gc3[:, lo:hi, :], params_d[:],
                itc[:, lo * 8:hi * 8], n, n, HIDDEN, single_packet=False)

        # --- BCE: DMAs, Exps (one table set) and DVE t*x ---
        ex_tiles = []
        for i in range(NBCE):
            b, j = divmod(i, NCH)
            sl = slice(j * CHUNK, (j + 1) * CHUNK)
            lt = io_pool.tile([128, CHUNK], mybir.dt.float8e4, tag="lt")
            nc.sync.dma_start(out=lt[:], in_=logits_d[b, :, sl])
            tt = io_pool.tile([128, CHUNK], mybir.dt.float8e4, tag="tt")
            nc.scalar.dma_start(out=tt[:], in_=targets_d[b, :, sl])
            ex = ex_pool.tile([128, CHUNK], mybir.dt.bfloat16, tag=f"ex{i}")
            nc.scalar.activation(out=ex[:], in_=lt[:], func=AF.Exp)
            ex_tiles.append(ex)
            prod = act_pool.tile([128, CHUNK], mybir.dt.bfloat16, tag="prod")
            nc.vector.tensor_tensor(out=prod[:], in0=lt[:], in1=tt[:],
                                    op=mybir.AluOpType.mult)
            nc.vector.tensor_scalar(
                out=prod[:], in0=prod[:], scalar1=1.0, scalar2=None,
                op0=mybir.AluOpType.mult, op1=mybir.AluOpType.add,
                accum_out=parts[:, NBCE + i:NBCE + i + 1])

        # --- BCE: Lns (second table set, loaded once) ---
        for i in range(NBCE):
            sp = act_pool.tile([128, CHUNK], mybir.dt.float8e4, tag="sp")
            nc.scalar.activation(out=sp[:], in_=ex_tiles[i][:], func=AF.Ln,
                                 bias=1.0, accum_out=parts[:, i:i + 1])

        # --- regularizer: subtract on GPSIMD (fp8 in/out), sum(d^2) split ---
        seg = RCOLS * HIDDEN
        for r in range(RCH):
            sl = slice(r * seg, (r + 1) * seg)
            d = d_pool.tile([128, seg], mybir.dt.float8e4, tag="d")
            nc.gpsimd.tensor_tensor(out=d[:], in0=gp[:, sl], in1=gc[:, sl],
                                    op=mybir.AluOpType.subtract)
            if r < N_DVE_SQ:
                dsq = d_pool.tile([128, seg], mybir.dt.bfloat16, tag="dsq")
                nc.vector.tensor_tensor(out=dsq[:], in0=d[:], in1=d[:],
                                        op=mybir.AluOpType.mult)
                nc.vector.tensor_scalar(
                    out=dsq[:], in0=dsq[:], scalar1=1.0, scalar2=None,
                    op0=mybir.AluOpType.mult, op1=mybir.AluOpType.add,
                    accum_out=parts[:, 2 * NBCE + r:2 * NBCE + r + 1])
            else:
                asq = d_pool.tile([128, seg], mybir.dt.float8e4, tag="asq")
                nc.scalar.activation(out=asq[:], in_=d[:], func=AF.Square,
                                     accum_out=parts[:, 2 * NBCE + r:2 * NBCE + r + 1])

        nc.sync.dma_start(out=out_d[:], in_=parts[:])
    nc.compile()
    return nc


def _wrap_idxs(idxs):
    """[N] ints -> [128, N/16] int16 dma_gather layout: idx i at [i%16, i//16],
    rows replicated 8x down the 128 partitions."""
    n = idxs.size
    a = np.zeros((16, n // 16), np.int16)
    a[np.arange(n) % 16, np.arange(n) // 16] = idxs.astype(np.int16)
    return np.tile(a, (8, 1))


def _get_nc():
    if "nc" not in _cache:
        _cache["nc"] = _build_nc()
    return _cache["nc"]


def make_in_maps(logits, targets, params, parent_idx, child_idx):
    lb = logits.astype(fp8).reshape(N_CORES, BLOCKS, 128, N_LABELS)
    tb = targets.astype(fp8).reshape(N_CORES, BLOCKS, 128, N_LABELS)
    pb = params.astype(fp8)
    in_maps = []
    for c in range(N_CORES):
        pe = parent_idx[c * EDGES_PC:(c + 1) * EDGES_PC].astype(np.int64)
        ce = child_idx[c * EDGES_PC:(c + 1) * EDGES_PC].astype(np.int64)
        order = np.argsort(pe, kind="stable")  # HBM locality for parent gather
        pe, ce = pe[order], ce[order]
        pad = EDGES_PAD - EDGES_PC
        pe = np.concatenate([pe, np.zeros(pad, np.int64)])
        ce = np.concatenate([ce, np.zeros(pad, np.int64)])
        in_maps.append({
            "logits": lb[c], "targets": tb[c], "params": pb,
            "idxp": _wrap_idxs(pe), "idxc": _wrap_idxs(ce),
        })
    return in_maps


def reduce_partials(partials_list):
    p = np.stack([np.asarray(x, dtype=np.float64) for x in partials_list])
    sp_sum = p[:, :, 0:NBCE].sum()
    tx_sum = p[:, :, NBCE:2 * NBCE].sum()
    reg_sum = p[:, :, 2 * NBCE:2 * NBCE + RCH].sum()
    bce = (sp_sum - tx_sum) / (BATCH * N_LABELS)
    loss = bce + PENALTY * 0.5 * reg_sum
    return np.asarray(loss, dtype=np.float32)


def kernel(logits, targets, params, parent_idx, child_idx):
    nc = _get_nc()
    in_maps = make_in_maps(logits, targets, params, parent_idx, child_idx)
    res = run_bass_kernel_spmd(nc, in_maps, list(range(N_CORES)))
    return reduce_partials([r["partials"] for r in res.results])


if __name__ == "__main__":
    rng = np.random.default_rng(0)
    out = kernel(
        rng.standard_normal((BATCH, N_LABELS)).astype(np.float32),
        rng.random((BATCH, N_LABELS)).astype(np.float32),
        rng.standard_normal((N_LABELS, HIDDEN)).astype(np.float32),
        rng.integers(0, N_LABELS, N_EDGES).astype(np.int32),
        rng.integers(0, N_LABELS, N_EDGES).astype(np.int32),
    )
    print("loss:", out, out.shape, out.dtype)


# revision 3
# speedup vs baseline: 1.1791x; 1.1791x over previous
"""Trainium2 Bass kernel v7 for nn_ClassificationLoss (BCE-with-logits +
graph Laplacian regularizer), data-parallel over 8 NeuronCores.

loss = mean(softplus(logits) - targets*logits)
       + 1e-4 * 0.5 * sum_e ||params[parent_e] - params[child_e]||^2

Design (per core, all inputs fp8 e4m3 converted on host; HW-verified op set:
tensor_tensor fp8 on DVE, tensor_scalar+accum, GPSIMD tensor_tensor with
matching in/out dtype, fp8 dma_gather; tensor_tensor_reduce is NOT used --
it faults this hardware):
  - BCE rows [256c, 256c+256): 4 chunks of [128, 5000]. softplus via ACT
    Exp then Ln(bias=1, accum_out), all Exps grouped before all Lns so the
    activation-table loads collapse to one per set. t*x = DVE
    tensor_tensor mult (fp8 -> bf16) + tensor_scalar mult-by-1 with
    accum_out (2x on the bf16 product).
  - Regularizer edges [2500c, 2500c+2500) padded to 2560, gathered fp8 in
    two chunks per side (4+16 of 20 gather columns). d = gp - gc on GPSIMD
    (fp8 in/out -- dtype conversion on GPSIMD faults; HW-measured ~0.8us
    per chunk, 7x faster than the cost model). sum(d^2) entirely on DVE
    (mult + ts-accum) -- ACT is the measured hardware bottleneck, so no
    Squares run there.
  - Partial sums land in a [128, 16] f32 tensor, host-reduced in f64.
"""
import os
import sys

import numpy as np
import ml_dtypes

for _p in ("/opt/trn_rl_repo", "/root/.axon_site/_ro/trn_rl_repo"):
    if os.path.isdir(_p) and _p not in sys.path:
        sys.path.append(_p)

from contextlib import ExitStack

import concourse.bass as bass
import concourse.tile as tile
from concourse import bacc, mybir
from concourse.bass_utils import run_bass_kernel_spmd

fp8 = ml_dtypes.float8_e4m3
AF = mybir.ActivationFunctionType

N_CORES = 8
BATCH, N_LABELS, HIDDEN, N_EDGES = 2048, 10000, 768, 20000
PENALTY = 1e-4
ROWS = BATCH // N_CORES            # 256 rows per core
BLOCKS = ROWS // 128               # 2 partition blocks
NCH = 2                            # bce col-chunks per block
CHUNK = N_LABELS // NCH            # 5000 (640 KB per fp8 DMA)
EDGES_PC = N_EDGES // N_CORES      # 2500 edges per core
EDGES_PAD = 2560                   # padded to 20*128
GCOLS = EDGES_PAD // 128           # 20 gather columns
GSPLIT = 4                         # gather chunk 0 covers cols [0,4)
RCH = 5                            # reg compute chunks (4 cols each)
RCOLS = GCOLS // RCH
N_DVE_SQ = 5                       # reg chunks whose square runs on DVE
NBCE = BLOCKS * NCH                # 4 bce chunks
# partials columns: [0:4) softplus sums, [4:8) t*x sums, [8:13) reg sums
P_COLS = 16

_cache = {}


def _build_nc():
    nc = bacc.Bacc("TRN2", target_bir_lowering=False, debug=False,
                   num_devices=N_CORES)
    with tile.TileContext(nc) as tc, ExitStack() as ctx:
        io_pool = ctx.enter_context(tc.tile_pool(name="io", bufs=4))
        act_pool = ctx.enter_context(tc.tile_pool(name="act", bufs=2))
        ex_pool = ctx.enter_context(tc.tile_pool(name="ex", bufs=1))
        g_pool = ctx.enter_context(tc.tile_pool(name="g", bufs=1))
        d_pool = ctx.enter_context(tc.tile_pool(name="d", bufs=4))

        logits_d = nc.dram_tensor(
            "logits", [BLOCKS, 128, N_LABELS], mybir.dt.float8e4, kind="ExternalInput")
        targets_d = nc.dram_tensor(
            "targets", [BLOCKS, 128, N_LABELS], mybir.dt.float8e4, kind="ExternalInput")
        params_d = nc.dram_tensor(
            "params", [N_LABELS, HIDDEN], mybir.dt.float8e4, kind="ExternalInput")
        idxp_d = nc.dram_tensor(
            "idxp", [128, EDGES_PAD // 16], mybir.dt.int16, kind="ExternalInput")
        idxc_d = nc.dram_tensor(
            "idxc", [128, EDGES_PAD // 16], mybir.dt.int16, kind="ExternalInput")
        out_d = nc.dram_tensor(
            "partials", [128, P_COLS], mybir.dt.float32, kind="ExternalOutput")

        parts = g_pool.tile([128, P_COLS], mybir.dt.float32)
        nc.vector.memset(parts[:], 0.0)

        # edge index loads first on the scalar ring (small)
        itp = g_pool.tile([128, EDGES_PAD // 16], mybir.dt.int16)
        itc = g_pool.tile([128, EDGES_PAD // 16], mybir.dt.int16)
        nc.scalar.dma_start(out=itp[:], in_=idxp_d[:])
        nc.scalar.dma_start(out=itc[:], in_=idxc_d[:])

        # --- regularizer gathers: 2 chunks per side (SWDGE) ---
        gp = g_pool.tile([128, GCOLS * HIDDEN], mybir.dt.float8e4)
        gc = g_pool.tile([128, GCOLS * HIDDEN], mybir.dt.float8e4)
        gp3 = gp[:].rearrange("p (c s) -> p c s", s=HIDDEN)
        gc3 = gc[:].rearrange("p (c s) -> p c s", s=HIDDEN)
        for lo, hi in ((0, GSPLIT), (GSPLIT, GCOLS)):
            n = (hi - lo) * 128
            nc.gpsimd.dma_gather(
                gp3[:, lo:hi, :], params_d[:],
                itp[:, lo * 8:hi * 8], n, n, HIDDEN, single_packet=False)
            nc.gpsimd.dma_gather(
                gc3[:, lo:hi, :], params_d[:],
                itc[:, lo * 8:hi * 8], n, n, HIDDEN, single_packet=False)

        # --- BCE: DMAs, Exps (one table set) and DVE t*x ---
        ex_tiles = []
        for i in range(NBCE):
            b, j = divmod(i, NCH)
            sl = slice(j * CHUNK, (j + 1) * CHUNK)
            lt = io_pool.tile([128, CHUNK], mybir.dt.float8e4, tag="lt")
            nc.sync.dma_start(out=lt[:], in_=logits_d[b, :, sl])
            tt = io_pool.tile([128, CHUNK], mybir.dt.float8e4, tag="tt")
            nc.scalar.dma_start(out=tt[:], in_=targets_d[b, :, sl])
            ex = ex_pool.tile([128, CHUNK], mybir.dt.bfloat16, tag=f"ex{i}")
            nc.scalar.activation(out=ex[:], in_=lt[:], func=AF.Exp)
            ex_tiles.append(ex)
            prod = act_pool.tile([128, CHUNK], mybir.dt.bfloat16, tag="prod")
            nc.vector.tensor_tensor(out=prod[:], in0=lt[:], in1=tt[:],
                                    op=mybir.AluOpType.mult)
            nc.vector.tensor_scalar(
                out=prod[:], in0=prod[:], scalar1=1.0, scalar2=None,
                op0=mybir.AluOpType.mult, op1=mybir.AluOpType.add,
                accum_out=parts[:, NBCE + i:NBCE + i + 1])

        # --- BCE: Lns (second table set, loaded once) ---
        for i in range(NBCE):
            sp = act_pool.tile([128, CHUNK], mybir.dt.float8e4, tag="sp")
            nc.scalar.activation(out=sp[:], in_=ex_tiles[i][:], func=AF.Ln,
                                 bias=1.0, accum_out=parts[:, i:i + 1])

        # --- regularizer: subtract on GPSIMD (fp8 in/out), sum(d^2) split ---
        seg = RCOLS * HIDDEN
        for r in range(RCH):
            sl = slice(r * seg, (r + 1) * seg)
            d = d_pool.tile([128, seg], mybir.dt.float8e4, tag="d")
            nc.gpsimd.tensor_tensor(out=d[:], in0=gp[:, sl], in1=gc[:, sl],
                                    op=mybir.AluOpType.subtract)
            if r < N_DVE_SQ:
                dsq = d_pool.tile([128, seg], mybir.dt.bfloat16, tag="dsq")
                nc.vector.tensor_tensor(out=dsq[:], in0=d[:], in1=d[:],
                                        op=mybir.AluOpType.mult)
                nc.vector.tensor_scalar(
                    out=dsq[:], in0=dsq[:], scalar1=1.0, scalar2=None,
                    op0=mybir.AluOpType.mult, op1=mybir.AluOpType.add,
                    accum_out=parts[:, 2 * NBCE + r:2 * NBCE + r + 1])
            else:
                asq = d_pool.tile([128, seg], mybir.dt.float8e4, tag="asq")
                nc.scalar.activation(out=asq[:], in_=d[:], func=AF.Square,
                                     accum_out=parts[:, 2 * NBCE + r:2 * NBCE + r + 1])

        nc.sync.dma_start(out=out_d[:], in_=parts[:])
    nc.compile()
    return nc


def _wrap_idxs(idxs):
    """[N] ints -> [128, N/16] int16 dma_gather layout: idx i at [i%16, i//16],
    rows replicated 8x down the 128 partitions."""
    n = idxs.size
    a = np.zeros((16, n // 16), np.int16)
    a[np.arange(n) % 16, np.arange(n) // 16] = idxs.astype(np.int16)
    return np.tile(a, (8, 1))


def _get_nc():
    if "nc" not in _cache:
        _cache["nc"] = _build_nc()
    return _cache["nc"]


def make_in_maps(logits, targets, params, parent_idx, child_idx):
    lb = logits.astype(fp8).reshape(N_CORES, BLOCKS, 128, N_LABELS)
    tb = targets.astype(fp8).reshape(N_CORES, BLOCKS, 128, N_LABELS)
    pb = params.astype(fp8)
    in_maps = []
    for c in range(N_CORES):
        pe = parent_idx[c * EDGES_PC:(c + 1) * EDGES_PC].astype(np.int64)
        ce = child_idx[c * EDGES_PC:(c + 1) * EDGES_PC].astype(np.int64)
        order = np.argsort(pe, kind="stable")  # HBM locality for parent gather
        pe, ce = pe[order], ce[order]
        pad = EDGES_PAD - EDGES_PC
        pe = np.concatenate([pe, np.zeros(pad, np.int64)])
        ce = np.concatenate([ce, np.zeros(pad, np.int64)])
        in_maps.append({
            "logits": lb[c], "targets": tb[c], "params": pb,
            "idxp": _wrap_idxs(pe), "idxc": _wrap_idxs(ce),
        })
    return in_maps


def reduce_partials(partials_list):
    p = np.stack([np.asarray(x, dtype=np.float64) for x in partials_list])
    sp_sum = p[:, :, 0:NBCE].sum()
    tx_sum = p[:, :, NBCE:2 * NBCE].sum()
    reg_sum = p[:, :, 2 * NBCE:2 * NBCE + RCH].sum()
    bce = (sp_sum - tx_sum) / (BATCH * N_LABELS)
    loss = bce + PENALTY * 0.5 * reg_sum
    return np.asarray(loss, dtype=np.float32)


def kernel(logits, targets, params, parent_idx, child_idx):
    nc = _get_nc()
    in_maps = make_in_maps(logits, targets, params, parent_idx, child_idx)
    res = run_bass_kernel_spmd(nc, in_maps, list(range(N_CORES)))
    return reduce_partials([r["partials"] for r in res.results])


if __name__ == "__main__":
    rng = np.random.default_rng(0)
    out = kernel(
        rng.standard_normal((BATCH, N_LABELS)).astype(np.float32),
        rng.random((BATCH, N_LABELS)).astype(np.float32),
        rng.standard_normal((N_LABELS, HIDDEN)).astype(np.float32),
        rng.integers(0, N_LABELS, N_EDGES).astype(np.int32),
        rng.integers(0, N_LABELS, N_EDGES).astype(np.int32),
    )
    print("loss:", out, out.shape, out.dtype)
